# revision 10
# baseline (speedup 1.0000x reference)
"""Canny edge-detection Bass kernel (per-core program), v3.

Geometry (per core):
  - Output rows: rows_out (2048) of the tall image, [R0, R0+rows_out).
  - Tile t reads input rows [120t, 120t+128) of the xs shard (xs row 0 is
    tall row R0-6); valid NMS rows on partitions p in [2, 122).
  - Tiles are processed in groups of G=3: elementwise stages run once per
    group on [128, G*N] tensors (strided 3-d APs per sub-tile), amortizing
    the per-instruction fixed cost (~131ns DVE / ~185ns Act / ~156ns Pool).

Pipeline (engine placement from measured cost-model + compiler support):
  - Quant (per tile): tf=(x+1)*127.5 (Pool), rf=round via 2^23 (Pool, bf16
    out), fixb=rf>tf (DVE mixed), u=rf-fixb (DVE bf16 2x) = floor exact.
  - Sobel fully on PE: gx = Sb@u[2]-Sb@u[0], gy = Sv@u[0]+2Sv@u[1]+Sv@u[2]
    accumulated in PSUM per channel; Act evacuates gx/gy (copy) and
    |gx|/|gy| (AF.Abs) into group tensors.
  - NMS: keep = mag > max(nb, na-1) (integer mags), sector maxes selected
    by 3 copy_predicated; km = mag*keep feeds both thresholds (Pool ts).
  - Hysteresis: 18 net rows + 3-bit margins per int32 word (24 bits, f32
    pack exact); 3 iterations are word-local (no partition-shift DMAs).
    Unpack with per-out-tile stationaries, OUT_TILE=128.
"""
import sys
sys.path.insert(0, '/opt/trn_rl_repo')
from contextlib import ExitStack
import numpy as np
import ml_dtypes

import concourse.bass as bass
import concourse.tile as tile
from concourse import bacc, mybir

F32 = mybir.dt.float32
BF16 = mybir.dt.bfloat16
I16 = mybir.dt.int16
I32 = mybir.dt.int32

OP = mybir.AluOpType
AF = mybir.ActivationFunctionType

TAN22 = 0.4142135623730951
TAN67 = 2.414213562373095

STRIDE = 120          # valid mask rows per tile
TILE_R = 128          # input rows per tile
MPACK = 18            # net rows per packed int32 word
MARG = 3              # margin bits each side of the net range
OUT_TILE = 128        # output rows per unpack tile
G = 3                 # tiles per elementwise group

BF = ml_dtypes.bfloat16


def ext_rows(T):
    return STRIDE * (T - 1) + TILE_R  # xs shard rows


def make_consts(T=18, rows_out=2048):
    WORDS = (STRIDE * T) // MPACK     # 2160/18 = 120
    n_out = rows_out // OUT_TILE      # 16
    # Sobel vertical stationaries, lhsT layout: out[m] = sum_k lhsT[k,m] u[k]
    # blocks: [S_blur | -S_blur | S_vd | 2*S_vd]
    sob = np.zeros((128, 512), np.float32)
    for m in range(128):
        sob[m, m] = 2.0
        if m - 1 >= 0:
            sob[m - 1, m] = 1.0
        if m + 1 < 128:
            sob[m + 1, m] = 1.0
        if m + 1 < 128:
            sob[m + 1, 256 + m] = 1.0
        if m - 1 >= 0:
            sob[m - 1, 256 + m] = -1.0
    sob[:, 128:256] = -sob[:, 0:128]
    sob[:, 384:512] = 2.0 * sob[:, 256:384]
    # margin-pack stationaries: strip row s lands in every word w where
    # bit b = s - 18w + 3 is in [0, 24); net bits are [3, 21).
    # (built per-core in shard_inputs to zero out-of-image rows)
    # unpack one-hots, per out-tile: partition p reads strip row
    # s = 4 + 128o + p -> word w = s//18 (8-row window from w0(o)),
    # bit b = s%18 + 3, byte j = b//8, in-byte bit k = b%8.
    mrep = np.zeros((8, n_out * 3 * 128), np.float32)
    patc = np.zeros((128, n_out), np.int32)
    w0s = []
    for o in range(n_out):
        w0 = (4 + OUT_TILE * o) // MPACK
        w0s.append(w0)
        for p in range(128):
            s = 4 + OUT_TILE * o + p
            w, b = s // MPACK, s % MPACK + MARG
            j, k = b // 8, b % 8
            assert 0 <= w - w0 < 8
            mrep[w - w0, (o * 3 + j) * 128 + p] = 1.0
            patc[p, o] = 1 << k
    return {"sob": sob.astype(BF), "mrep": mrep.astype(BF),
            "patc": patc}, w0s


def make_p24(T, WORDS, valid):
    """Pack stationary [128, T*WORDS]; valid[t, p] gates strip rows."""
    p24 = np.zeros((128, T * WORDS), np.float32)
    for t in range(T):
        for p in range(2, 122):
            if not valid[t, p]:
                continue
            s = STRIDE * t + (p - 2)
            for w in range(WORDS):
                b = s - MPACK * w + MARG
                if 0 <= b < MPACK + 2 * MARG:
                    p24[p, t * WORDS + w] = float(1 << b)
    return p24


def build_canny(T=18, rows_out=2048, hyst_iters=3):
    EXT = ext_rows(T)
    WORDS = (STRIDE * T) // MPACK
    assert STRIDE * T % MPACK == 0 and WORDS <= 128
    n_out = rows_out // OUT_TILE
    NG = T // G
    assert T % G == 0

    nc = bacc.Bacc("TRN2", target_bir_lowering=False, debug=False,
                   num_devices=8)
    xs = nc.dram_tensor("xs", [3, EXT, 514], F32, kind="ExternalInput").ap()
    sob = nc.dram_tensor("sob", [128, 512], BF16, kind="ExternalInput").ap()
    p24 = nc.dram_tensor("p24", [128, T * WORDS], BF16,
                         kind="ExternalInput").ap()
    mrep = nc.dram_tensor("mrep", [8, n_out * 3 * 128], BF16,
                          kind="ExternalInput").ap()
    patc = nc.dram_tensor("patc", [128, n_out], I32,
                          kind="ExternalInput").ap()
    rvk = nc.dram_tensor("rvk", [128, 2 * 514], I16,
                         kind="ExternalInput").ap()
    out = nc.dram_tensor("out", [rows_out, 512], F32,
                         kind="ExternalOutput").ap()

    NE = 3 * 512   # evac cols per tile
    NQ = 3 * 514   # quant cols per tile

    with tile.TileContext(nc) as tc:
        with ExitStack() as octx:
            cpool = octx.enter_context(tc.tile_pool(name="consts", bufs=1))
            sob_b = cpool.tile([128, 512], BF16, tag="sobb")
            nc.sync.dma_start(sob_b[:], sob[:, :])
            p24_b = cpool.tile([128, T * WORDS], BF16, tag="p24b")
            nc.sync.dma_start(p24_b[:], p24[:, :])
            mrep_b = cpool.tile([8, n_out * 3 * 128], BF16, tag="mrepb")
            nc.sync.dma_start(mrep_b[:], mrep[:, :])
            patc_s = cpool.tile([128, n_out], I32, tag="patcs")
            nc.sync.dma_start(patc_s[:], patc[:, :])
            rvk_s = cpool.tile([128, 2 * 514], I16, tag="rvks")
            nc.sync.dma_start(rvk_s[:], rvk[:, :])

            pk = octx.enter_context(
                tc.tile_pool(name="packps", bufs=1, space="PSUM"))
            mmS = pk.tile([WORDS, 512], F32, tag="mmS")
            mmW = pk.tile([WORDS, 512], F32, tag="mmW")

            # ============ phase A: grouped Sobel + NMS ============
            with ExitStack() as actx:
                xin_p = actx.enter_context(tc.tile_pool(name="xin", bufs=3))
                qf_p = actx.enter_context(tc.tile_pool(name="qf", bufs=2))
                qb_p = actx.enter_context(tc.tile_pool(name="qb", bufs=3))
                pgx = actx.enter_context(
                    tc.tile_pool(name="pgx", bufs=1, space="PSUM"))
                pgy = actx.enter_context(
                    tc.tile_pool(name="pgy", bufs=1, space="PSUM"))
                ev_p = actx.enter_context(tc.tile_pool(name="ev", bufs=1))
                mgp = actx.enter_context(tc.tile_pool(name="mgp", bufs=1))
                sml = actx.enter_context(tc.tile_pool(name="sml", bufs=1))
                swp = actx.enter_context(tc.tile_pool(name="swp", bufs=2))

                for g in range(NG):
                    gxG = ev_p.tile([128, G * NE], I16, tag="gxG",
                                    name=f"gxG{g}")
                    gyG = ev_p.tile([128, G * NE], I16, tag="gyG",
                                    name=f"gyG{g}")
                    agxG = ev_p.tile([128, G * NE], I16, tag="agxG",
                                     name=f"agxG{g}")
                    agyG = ev_p.tile([128, G * NE], I16, tag="agyG",
                                     name=f"agyG{g}")
                    for k in range(G):
                        t = g * G + k
                        a = STRIDE * t
                        xin = xin_p.tile([128, NQ], F32, tag="xin",
                                         name=f"xin{t}")
                        for c in range(3):
                            nc.sync.dma_start(
                                xin[:, c * 514:(c + 1) * 514],
                                xs[c, a:a + 128, :])
                        tf = qf_p.tile([128, NQ], F32, tag="tf",
                                       name=f"tf{t}")
                        nc.gpsimd.tensor_scalar(
                            out=tf[:], in0=xin[:], scalar1=1.0,
                            scalar2=127.5, op0=OP.add, op1=OP.mult)
                        rf = qb_p.tile([128, NQ], BF16, tag="rf",
                                       name=f"rf{t}")
                        nc.gpsimd.tensor_scalar(
                            out=rf[:], in0=tf[:], scalar1=float(2 ** 23),
                            scalar2=float(2 ** 23), op0=OP.add,
                            op1=OP.subtract)
                        fixb = qb_p.tile([128, NQ], BF16, tag="fixb",
                                         name=f"fixb{t}")
                        nc.vector.tensor_tensor(out=fixb[:], in0=rf[:],
                                                in1=tf[:], op=OP.is_gt)
                        u = qb_p.tile([128, NQ], BF16, tag="u", name=f"u{t}")
                        nc.vector.tensor_tensor(out=u[:], in0=rf[:],
                                                in1=fixb[:], op=OP.subtract)
                        gxP = pgx.tile([128, NE], F32, tag="gxP",
                                       name=f"gxP{t}")
                        gyP = pgy.tile([128, NE], F32, tag="gyP",
                                       name=f"gyP{t}")
                        for c in range(3):
                            o = c * 514
                            d = gxP[:, c * 512:(c + 1) * 512]
                            nc.tensor.matmul(d, sob_b[:, 128:256],
                                             u[:, o:o + 512], start=True,
                                             stop=False)
                            nc.tensor.matmul(d, sob_b[:, 0:128],
                                             u[:, o + 2:o + 514],
                                             start=False, stop=True)
                            d = gyP[:, c * 512:(c + 1) * 512]
                            nc.tensor.matmul(d, sob_b[:, 256:384],
                                             u[:, o:o + 512], start=True,
                                             stop=False)
                            nc.tensor.matmul(d, sob_b[:, 384:512],
                                             u[:, o + 1:o + 513],
                                             start=False, stop=False)
                            nc.tensor.matmul(d, sob_b[:, 256:384],
                                             u[:, o + 2:o + 514],
                                             start=False, stop=True)
                        sl = slice(k * NE, (k + 1) * NE)
                        nc.scalar.copy(gxG[:, sl], gxP[:])
                        nc.scalar.copy(gyG[:, sl], gyP[:])
                        nc.scalar.activation(agxG[:, sl], gxP[:], AF.Abs)
                        nc.scalar.activation(agyG[:, sl], gyP[:], AF.Abs)

                    magcG = ev_p.tile([128, G * NE], I16, tag="magcG",
                                      name=f"magcG{g}")
                    nc.vector.tensor_tensor(out=magcG[:], in0=agxG[:],
                                            in1=agyG[:], op=OP.add)
                    # group views [128, G, .] per channel
                    mGv = magcG[:].rearrange("p (g n) -> p g n", g=G)
                    gxV = gxG[:].rearrange("p (g n) -> p g n", g=G)
                    gyV = gyG[:].rearrange("p (g n) -> p g n", g=G)
                    m0, m1, m2 = (mGv[:, :, c * 512:(c + 1) * 512]
                                  for c in range(3))
                    g0, g1, g2 = (gxV[:, :, c * 512:(c + 1) * 512]
                                  for c in range(3))
                    h0, h1, h2 = (gyV[:, :, c * 512:(c + 1) * 512]
                                  for c in range(3))
                    NS = G * 512
                    cmp01 = sml.tile([128, NS], I16, tag="cmp01",
                                     name=f"cmp01_{g}")
                    c01 = cmp01[:].rearrange("p (g n) -> p g n", g=G)
                    nc.vector.tensor_tensor(out=c01, in0=m0, in1=m1,
                                            op=OP.is_ge)
                    m01 = sml.tile([128, NS], I16, tag="m01", name=f"m01_{g}")
                    m01v = m01[:].rearrange("p (g n) -> p g n", g=G)
                    nc.vector.tensor_tensor(out=m01v, in0=m0, in1=m1,
                                            op=OP.max)
                    pick2 = sml.tile([128, NS], I16, tag="pick2",
                                     name=f"pick2_{g}")
                    p2v = pick2[:].rearrange("p (g n) -> p g n", g=G)
                    nc.vector.tensor_tensor(out=p2v, in0=m2, in1=m01v,
                                            op=OP.is_gt)
                    gxs = sml.tile([128, NS], I16, tag="gxs", name=f"gxs{g}")
                    gxsv = gxs[:].rearrange("p (g n) -> p g n", g=G)
                    nc.scalar.copy(gxsv, g1)
                    nc.vector.copy_predicated(gxsv, c01, g0)
                    nc.vector.copy_predicated(gxsv, p2v, g2)
                    gys = sml.tile([128, NS], I16, tag="gys", name=f"gys{g}")
                    gysv = gys[:].rearrange("p (g n) -> p g n", g=G)
                    nc.scalar.copy(gysv, h1)
                    nc.vector.copy_predicated(gysv, c01, h0)
                    nc.vector.copy_predicated(gysv, p2v, h2)
                    magp = mgp.tile([128, G * 514], I16, tag="magp",
                                    name=f"magp{g}")
                    mpv = magp[:].rearrange("p (g n) -> p g n", g=G)
                    nc.gpsimd.memset(magp[:], 0)
                    nc.vector.tensor_tensor(out=mpv[:, :, 1:513], in0=m01v,
                                            in1=m2, op=OP.max)
                    for bi_, t_ in ((0, 0), (1, T - 1)):
                        if t_ // G == g:
                            k_ = t_ % G
                            tmpb = mgp.tile([128, 514], I16, tag="tmpb",
                                            name=f"tmpb{g}")
                            nc.vector.tensor_tensor(
                                out=tmpb[:],
                                in0=magp[:, k_ * 514:(k_ + 1) * 514],
                                in1=rvk_s[:, bi_ * 514:(bi_ + 1) * 514],
                                op=OP.mult)
                            nc.vector.tensor_copy(
                                magp[:, k_ * 514:(k_ + 1) * 514], tmpb[:])
                    # sector masks
                    ax = sml.tile([128, NS], I16, tag="ax", name=f"ax{g}")
                    nc.scalar.activation(ax[:], gxs[:], AF.Abs)
                    ay = sml.tile([128, NS], I16, tag="ay", name=f"ay{g}")
                    nc.scalar.activation(ay[:], gys[:], AF.Abs)
                    hm = sml.tile([128, NS], I16, tag="hm", name=f"hm{g}")
                    nc.vector.scalar_tensor_tensor(
                        out=hm[:], in0=ax[:], scalar=TAN22, in1=ay[:],
                        op0=OP.mult, op1=OP.is_gt)
                    vm = sml.tile([128, NS], I16, tag="vm", name=f"vm{g}")
                    nc.vector.scalar_tensor_tensor(
                        out=vm[:], in0=ax[:], scalar=TAN67, in1=ay[:],
                        op0=OP.mult, op1=OP.is_lt)
                    pp = sml.tile([128, NS], BF16, tag="pp", name=f"pp{g}")
                    nc.vector.tensor_tensor(out=pp[:], in0=gxs[:],
                                            in1=gys[:], op=OP.mult)
                    ssm = sml.tile([128, NS], I16, tag="ssm", name=f"ssm{g}")
                    nc.gpsimd.tensor_scalar(out=ssm[:], in0=pp[:],
                                            scalar1=0.0, scalar2=None,
                                            op0=OP.is_ge)
                    # neighbors via partition-shift DMA (whole group)
                    mu = mgp.tile([128, G * 514], I16, tag="mu",
                                  name=f"mu{g}")
                    nc.gpsimd.memset(mu[96:128, :], 0)
                    nc.sync.dma_start(mu[0:127, :], magp[1:128, :])
                    md = mgp.tile([128, G * 514], I16, tag="md",
                                  name=f"md{g}")
                    nc.gpsimd.memset(md[0:32, :], 0)
                    nc.sync.dma_start(md[1:128, :], magp[0:127, :])
                    mum1 = mgp.tile([128, G * 514], I16, tag="mum1",
                                    name=f"mum1_{g}")
                    nc.gpsimd.tensor_scalar(out=mum1[:], in0=mu[:],
                                            scalar1=1, scalar2=None,
                                            op0=OP.subtract)
                    mgm1 = mgp.tile([128, G * 514], I16, tag="mgm1",
                                    name=f"mgm1_{g}")
                    nc.gpsimd.tensor_scalar(out=mgm1[:], in0=magp[:],
                                            scalar1=1, scalar2=None,
                                            op0=OP.subtract)
                    muv = mu[:].rearrange("p (g n) -> p g n", g=G)
                    mdv = md[:].rearrange("p (g n) -> p g n", g=G)
                    mu1v = mum1[:].rearrange("p (g n) -> p g n", g=G)
                    mg1v = mgm1[:].rearrange("p (g n) -> p g n", g=G)
                    # keep = mag > max(nb, na-1); na-side uses >= via -1
                    M = sml.tile([128, NS], I16, tag="M", name=f"M{g}")
                    Mv_ = M[:].rearrange("p (g n) -> p g n", g=G)
                    nc.vector.tensor_tensor(out=Mv_, in0=mdv[:, :, 2:514],
                                            in1=mu1v[:, :, 0:512], op=OP.max)
                    Md1 = sml.tile([128, NS], I16, tag="Md1", name=f"Md1_{g}")
                    Md1v = Md1[:].rearrange("p (g n) -> p g n", g=G)
                    nc.vector.tensor_tensor(out=Md1v, in0=mdv[:, :, 0:512],
                                            in1=mu1v[:, :, 2:514], op=OP.max)
                    Mvv = sml.tile([128, NS], I16, tag="Mvv", name=f"Mvv{g}")
                    Mvvv = Mvv[:].rearrange("p (g n) -> p g n", g=G)
                    nc.vector.tensor_tensor(out=Mvvv, in0=mdv[:, :, 1:513],
                                            in1=mu1v[:, :, 1:513], op=OP.max)
                    Mh = sml.tile([128, NS], I16, tag="Mh", name=f"Mh{g}")
                    Mhv = Mh[:].rearrange("p (g n) -> p g n", g=G)
                    nc.vector.tensor_tensor(out=Mhv, in0=mpv[:, :, 0:512],
                                            in1=mg1v[:, :, 2:514], op=OP.max)
                    nc.vector.copy_predicated(M[:], ssm[:], Md1[:])
                    nc.vector.copy_predicated(M[:], vm[:], Mvv[:])
                    nc.vector.copy_predicated(M[:], hm[:], Mh[:])
                    kc = sml.tile([128, NS], I16, tag="kc", name=f"kc{g}")
                    kcv = kc[:].rearrange("p (g n) -> p g n", g=G)
                    nc.vector.tensor_tensor(out=kcv, in0=mpv[:, :, 1:513],
                                            in1=Mv_, op=OP.is_gt)
                    km = sml.tile([128, NS], I16, tag="km", name=f"km{g}")
                    kmv = km[:].rearrange("p (g n) -> p g n", g=G)
                    nc.vector.tensor_tensor(out=kmv, in0=mpv[:, :, 1:513],
                                            in1=kcv, op=OP.mult)
                    strong = swp.tile([128, NS], BF16, tag="strong",
                                      name=f"strong{g}")
                    nc.gpsimd.tensor_scalar(out=strong[:], in0=km[:],
                                            scalar1=200.0, scalar2=None,
                                            op0=OP.is_gt)
                    weak = swp.tile([128, NS], BF16, tag="weak",
                                    name=f"weak{g}")
                    nc.gpsimd.tensor_scalar(out=weak[:], in0=km[:],
                                            scalar1=100.0, scalar2=None,
                                            op0=OP.is_gt)
                    for k in range(G):
                        t = g * G + k
                        lhs = p24_b[:, t * WORDS:(t + 1) * WORDS]
                        ssl = slice(k * 512, (k + 1) * 512)
                        nc.tensor.matmul(mmS[:], lhs, strong[:, ssl],
                                         start=(t == 0), stop=(t == T - 1))
                        nc.tensor.matmul(mmW[:], lhs, weak[:, ssl],
                                         start=(t == 0), stop=(t == T - 1))

            # ============ phase B: packed hysteresis (word-local) ============
            with ExitStack() as bctx:
                hw_ = bctx.enter_context(tc.tile_pool(name="hw", bufs=1))
                it_p = bctx.enter_context(tc.tile_pool(name="itp", bufs=2))
                sW = hw_.tile([WORDS, 512], I32, tag="sW")
                nc.vector.tensor_copy(sW[:], mmW[:])
                cur = hw_.tile([WORDS, 512], I32, tag="cur0")
                nc.vector.tensor_copy(cur[:], mmS[:])
                for it in range(hyst_iters):
                    sl = it_p.tile([WORDS, 512], I32, tag="sl",
                                   name=f"sl{it}")
                    nc.vector.tensor_scalar(
                        out=sl[:], in0=cur[:], scalar1=1, scalar2=None,
                        op0=OP.logical_shift_left)
                    sr = it_p.tile([WORDS, 512], I32, tag="sr",
                                   name=f"sr{it}")
                    nc.vector.tensor_scalar(
                        out=sr[:], in0=cur[:], scalar1=1, scalar2=None,
                        op0=OP.logical_shift_right)
                    o1 = it_p.tile([WORDS, 512], I32, tag="o1",
                                   name=f"o1_{it}")
                    nc.vector.tensor_tensor(out=o1[:], in0=sl[:],
                                            in1=sr[:], op=OP.bitwise_or)
                    vor = it_p.tile([WORDS, 512], I32, tag="vor",
                                    name=f"vor{it}")
                    nc.vector.tensor_tensor(out=vor[:], in0=o1[:],
                                            in1=cur[:], op=OP.bitwise_or)
                    q = it_p.tile([WORDS, 512], I32, tag="q", name=f"q{it}")
                    nc.vector.tensor_tensor(
                        out=q[:, 1:512], in0=vor[:, 0:511],
                        in1=vor[:, 1:512], op=OP.bitwise_or)
                    nc.vector.tensor_copy(q[:, 0:1], vor[:, 0:1])
                    r = it_p.tile([WORDS, 512], I32, tag="r", name=f"r{it}")
                    nc.vector.tensor_tensor(
                        out=r[:, 0:511], in0=q[:, 0:511],
                        in1=vor[:, 1:512], op=OP.bitwise_or)
                    nc.vector.tensor_copy(r[:, 511:512], q[:, 511:512])
                    ncur = hw_.tile([WORDS, 512], I32, tag=f"cur{it + 1}",
                                    name=f"ncur{it + 1}")
                    nc.vector.tensor_tensor(out=ncur[:], in0=r[:],
                                            in1=sW[:], op=OP.bitwise_and)
                    cur = ncur
                bi = []
                for j, (s1v, s2v, o0, o1v) in enumerate([
                        (255, None, OP.bitwise_and, None),
                        (8, 255, OP.logical_shift_right, OP.bitwise_and),
                        (16, 255, OP.logical_shift_right, OP.bitwise_and),
                ]):
                    x_ = hw_.tile([WORDS, 512], I32, tag=f"bi{j}",
                                  name=f"bi{j}")
                    if o1v is None:
                        nc.vector.tensor_scalar(
                            out=x_[:], in0=cur[:], scalar1=s1v,
                            scalar2=None, op0=o0)
                    else:
                        nc.vector.tensor_scalar(
                            out=x_[:], in0=cur[:], scalar1=s1v,
                            scalar2=s2v, op0=o0, op1=o1v)
                    bi.append(x_)
                b0 = hw_.tile([WORDS, 512], BF16, tag="b0")
                nc.scalar.copy(b0[:], bi[0][:])
                b1 = hw_.tile([WORDS, 512], BF16, tag="b1")
                nc.scalar.copy(b1[:], bi[1][:])
                b2 = hw_.tile([WORDS, 512], BF16, tag="b2")
                nc.scalar.copy(b2[:], bi[2][:])
                unp = bctx.enter_context(
                    tc.tile_pool(name="unp", bufs=2, space="PSUM"))
                uo_p = bctx.enter_context(tc.tile_pool(name="uo", bufs=3))
                consts_host, w0s = make_consts(T, rows_out)
                for o in range(n_out):
                    w0 = w0s[o]
                    bs = [uo_p.tile([8, 512], BF16, tag=f"bs{j}",
                                    name=f"bs{j}_{o}")
                          for j in range(3)]
                    for j, bsrc in enumerate((b0, b1, b2)):
                        nc.sync.dma_start(bs[j][:], bsrc[w0:w0 + 8, :])
                    ps = unp.tile([128, 512], F32, tag="ps", name=f"ps{o}")
                    for j in range(3):
                        nc.tensor.matmul(
                            ps[:],
                            mrep_b[:, (o * 3 + j) * 128:(o * 3 + j + 1) * 128],
                            bs[j][:], start=(j == 0), stop=(j == 2))
                    pse = uo_p.tile([128, 512], I32, tag="pse",
                                    name=f"pse{o}")
                    nc.scalar.copy(pse[:], ps[:])
                    bits = uo_p.tile([128, 512], I32, tag="bits",
                                     name=f"bits{o}")
                    nc.vector.tensor_scalar(out=bits[:], in0=pse[:],
                                            scalar1=patc_s[:, o:o + 1],
                                            scalar2=None,
                                            op0=OP.bitwise_and)
                    ot = uo_p.tile([128, 512], F32, tag="ot", name=f"ot{o}")
                    nc.vector.tensor_scalar(out=ot[:], in0=bits[:],
                                            scalar1=0, scalar2=255.0,
                                            op0=OP.not_equal, op1=OP.mult)
                    nc.sync.dma_start(
                        out[o * OUT_TILE:(o + 1) * OUT_TILE, :], ot[:])

    nc.compile()
    return nc


# ---------------- host-side helpers ----------------

def shard_inputs(x, T=18, rows_out=2048, n_cores=8):
    B, C, H, W = x.shape
    NR = B * H
    WORDS = (STRIDE * T) // MPACK
    tall = np.ascontiguousarray(x.transpose(1, 0, 2, 3).reshape(C, NR, W))
    tallp = np.pad(tall, ((0, 0), (0, 0), (1, 1)), mode='edge')
    EXT = ext_rows(T)
    consts, _ = make_consts(T, rows_out)
    maps = []
    for k in range(n_cores):
        r0 = k * rows_out - 6
        idx = np.clip(np.arange(r0, r0 + EXT), 0, NR - 1)
        shard = np.ascontiguousarray(tallp[:, idx, :])
        # per-core row-validity for boundary tiles (tall row in [0, NR))
        rvk = np.ones((128, 2 * 514), np.int16)
        for bi, t in ((0, 0), (1, T - 1)):
            rows = r0 + STRIDE * t + np.arange(128)
            bad = (rows < 0) | (rows >= NR)
            rvk[bad, bi * 514:(bi + 1) * 514] = 0
        # per-core pack stationary: zero strip rows outside the image
        valid = np.zeros((T, 128), bool)
        for t in range(T):
            g = k * rows_out - 4 + STRIDE * t + (np.arange(128) - 2)
            valid[t] = (g >= 0) & (g < NR)
        p24 = make_p24(T, WORDS, valid)
        m = {"xs": shard, "rvk": rvk, "p24": p24.astype(BF)}
        m.update(consts)
        maps.append(m)
    return maps


def assemble_output(results, B=32, H=512, W=512):
    outs = [r["out"] for r in results]
    tallout = np.concatenate(outs, axis=0)
    img = tallout.reshape(B, H, W)
    return np.broadcast_to(img[:, None], (B, 3, H, W))


# ---------------- harness entry point ----------------

_NC_CACHE = {}


def _get_nc():
    if "nc" not in _NC_CACHE:
        _NC_CACHE["nc"] = build_canny(T=18, rows_out=2048, hyst_iters=3)
    return _NC_CACHE["nc"]


def kernel(x):
    """Full-input entry point: x (32,3,512,512) f32 -> (32,3,512,512) f32."""
    from concourse.bass_utils import run_bass_kernel_spmd
    x = np.asarray(x, dtype=np.float32)
    nc = _get_nc()
    in_maps = shard_inputs(x, T=18, rows_out=2048, n_cores=8)
    res = run_bass_kernel_spmd(nc, in_maps, list(range(8)))
    out = assemble_output(res.results)
    return np.ascontiguousarray(out).astype(np.float32)


# revision 16
# speedup vs baseline: 1.1021x; 1.1021x over previous
"""Canny edge-detection Bass kernel (per-core program), v3.

Geometry (per core):
  - Output rows: rows_out (2048) of the tall image, [R0, R0+rows_out).
  - Tile t reads input rows [120t, 120t+128) of the xs shard (xs row 0 is
    tall row R0-6); valid NMS rows on partitions p in [2, 122).
  - Tiles are processed in groups of G=3: elementwise stages run once per
    group on [128, G*N] tensors (strided 3-d APs per sub-tile), amortizing
    the per-instruction fixed cost (~131ns DVE / ~185ns Act / ~156ns Pool).

Pipeline (engine placement from measured cost-model + compiler support):
  - Quant (per tile): tf=(x+1)*127.5 (Pool), rf=round via 2^23 (Pool, bf16
    out), fixb=rf>tf (DVE mixed), u=rf-fixb (DVE bf16 2x) = floor exact.
  - Sobel fully on PE: gx = Sb@u[2]-Sb@u[0], gy = Sv@u[0]+2Sv@u[1]+Sv@u[2]
    accumulated in PSUM per channel; Act evacuates gx/gy (copy) and
    |gx|/|gy| (AF.Abs) into group tensors.
  - NMS: keep = mag > max(nb, na-1) (integer mags), sector maxes selected
    by 3 copy_predicated; km = mag*keep feeds both thresholds (Pool ts).
  - Hysteresis: 18 net rows + 3-bit margins per int32 word (24 bits, f32
    pack exact); 3 iterations are word-local (no partition-shift DMAs).
    Unpack with per-out-tile stationaries, OUT_TILE=128.
"""
import sys
sys.path.insert(0, '/opt/trn_rl_repo')
from contextlib import ExitStack
import numpy as np
import ml_dtypes

import concourse.bass as bass
import concourse.tile as tile
from concourse import bacc, mybir

F32 = mybir.dt.float32
BF16 = mybir.dt.bfloat16
I16 = mybir.dt.int16
I32 = mybir.dt.int32

OP = mybir.AluOpType
AF = mybir.ActivationFunctionType

TAN22 = 0.4142135623730951
TAN67 = 2.414213562373095

STRIDE = 120          # valid mask rows per tile
TILE_R = 128          # input rows per tile
MPACK = 18            # net rows per packed int32 word
MARG = 3              # margin bits each side of the net range
OUT_TILE = 128        # output rows per unpack tile
G = 3                 # tiles per elementwise group

BF = ml_dtypes.bfloat16


def ext_rows(T):
    return STRIDE * (T - 1) + TILE_R  # xs shard rows


def make_consts(T=18, rows_out=2048):
    WORDS = (STRIDE * T) // MPACK     # 2160/18 = 120
    n_out = rows_out // OUT_TILE      # 16
    # Sobel vertical stationaries, lhsT layout: out[m] = sum_k lhsT[k,m] u[k]
    # blocks: [S_blur | -S_blur | S_vd | 2*S_vd]
    sob = np.zeros((128, 512), np.float32)
    for m in range(128):
        sob[m, m] = 2.0
        if m - 1 >= 0:
            sob[m - 1, m] = 1.0
        if m + 1 < 128:
            sob[m + 1, m] = 1.0
        if m + 1 < 128:
            sob[m + 1, 256 + m] = 1.0
        if m - 1 >= 0:
            sob[m - 1, 256 + m] = -1.0
    sob[:, 128:256] = -sob[:, 0:128]
    sob[:, 384:512] = 2.0 * sob[:, 256:384]
    # margin-pack stationaries: strip row s lands in every word w where
    # bit b = s - 18w + 3 is in [0, 24); net bits are [3, 21).
    # (built per-core in shard_inputs to zero out-of-image rows)
    # unpack one-hots, per out-tile: partition p reads strip row
    # s = 4 + 128o + p -> word w = s//18 (8-row window from w0(o)),
    # bit b = s%18 + 3, byte j = b//8, in-byte bit k = b%8.
    mrep = np.zeros((8, n_out * 3 * 128), np.float32)
    patc = np.zeros((128, n_out), np.int32)
    w0s = []
    for o in range(n_out):
        w0 = (4 + OUT_TILE * o) // MPACK
        w0s.append(w0)
        for p in range(128):
            s = 4 + OUT_TILE * o + p
            w, b = s // MPACK, s % MPACK + MARG
            j, k = b // 8, b % 8
            assert 0 <= w - w0 < 8
            mrep[w - w0, (o * 3 + j) * 128 + p] = 1.0
            patc[p, o] = 1 << k
    return {"sob": sob.astype(BF), "mrep": mrep.astype(BF),
            "patc": patc}, w0s


def make_p24(T, WORDS, valid):
    """Pack stationary [128, T*WORDS]; valid[t, p] gates strip rows."""
    p24 = np.zeros((128, T * WORDS), np.float32)
    for t in range(T):
        for p in range(2, 122):
            if not valid[t, p]:
                continue
            s = STRIDE * t + (p - 2)
            for w in range(WORDS):
                b = s - MPACK * w + MARG
                if 0 <= b < MPACK + 2 * MARG:
                    p24[p, t * WORDS + w] = float(1 << b)
    return p24


def build_canny(T=18, rows_out=2048, hyst_iters=3):
    EXT = ext_rows(T)
    WORDS = (STRIDE * T) // MPACK
    assert STRIDE * T % MPACK == 0 and WORDS <= 128
    n_out = rows_out // OUT_TILE
    NG = T // G
    assert T % G == 0

    nc = bacc.Bacc("TRN2", target_bir_lowering=False, debug=False,
                   num_devices=8)
    xs = nc.dram_tensor("xs", [3, EXT, 514], F32, kind="ExternalInput").ap()
    sob = nc.dram_tensor("sob", [128, 512], BF16, kind="ExternalInput").ap()
    p24 = nc.dram_tensor("p24", [128, T * WORDS], BF16,
                         kind="ExternalInput").ap()
    mrep = nc.dram_tensor("mrep", [8, n_out * 3 * 128], BF16,
                          kind="ExternalInput").ap()
    patc = nc.dram_tensor("patc", [128, n_out], I32,
                          kind="ExternalInput").ap()
    rvk = nc.dram_tensor("rvk", [128, 2 * 514], I16,
                         kind="ExternalInput").ap()
    out = nc.dram_tensor("out", [rows_out, 512], F32,
                         kind="ExternalOutput").ap()

    NE = 3 * 512   # evac cols per tile
    NQ = 3 * 514   # quant cols per tile

    with tile.TileContext(nc) as tc:
        with ExitStack() as octx:
            cpool = octx.enter_context(tc.tile_pool(name="consts", bufs=1))
            sob_b = cpool.tile([128, 512], BF16, tag="sobb")
            nc.sync.dma_start(sob_b[:], sob[:, :])
            # remaining consts are DMA'd after tile 0's input loads (p24,
            # rvk) or at phase B start (mrep, patc) to keep the first
            # compute off the critical path.
            p24_b = cpool.tile([128, T * WORDS], BF16, tag="p24b")
            mrep_b = cpool.tile([8, n_out * 3 * 128], BF16, tag="mrepb")
            patc_s = cpool.tile([128, n_out], I32, tag="patcs")
            rvk_s = cpool.tile([128, 2 * 514], I16, tag="rvks")

            pk = octx.enter_context(
                tc.tile_pool(name="packps", bufs=1, space="PSUM"))
            mmS = pk.tile([WORDS, 512], F32, tag="mmS")
            mmW = pk.tile([WORDS, 512], F32, tag="mmW")

            # ============ phase A: grouped Sobel + NMS ============
            with ExitStack() as actx:
                xin_p = actx.enter_context(tc.tile_pool(name="xin", bufs=3))
                qf_p = actx.enter_context(tc.tile_pool(name="qf", bufs=2))
                qb_p = actx.enter_context(tc.tile_pool(name="qb", bufs=2))
                pgx = actx.enter_context(
                    tc.tile_pool(name="pgx", bufs=1, space="PSUM"))
                pgy = actx.enter_context(
                    tc.tile_pool(name="pgy", bufs=1, space="PSUM"))
                ev_p = actx.enter_context(tc.tile_pool(name="ev", bufs=2))
                mgp = actx.enter_context(tc.tile_pool(name="mgp", bufs=1))
                sml = actx.enter_context(tc.tile_pool(name="sml", bufs=1))
                swp = actx.enter_context(tc.tile_pool(name="swp", bufs=1))

                for g in range(NG):
                    gxG = ev_p.tile([128, G * NE], I16, tag="gxG",
                                    name=f"gxG{g}")
                    gyG = ev_p.tile([128, G * NE], I16, tag="gyG",
                                    name=f"gyG{g}")
                    agxG = ev_p.tile([128, G * NE], I16, tag="agxG",
                                     name=f"agxG{g}")
                    agyG = ev_p.tile([128, G * NE], I16, tag="agyG",
                                     name=f"agyG{g}")
                    for k in range(G):
                        t = g * G + k
                        a = STRIDE * t
                        xin = xin_p.tile([128, NQ], F32, tag="xin",
                                         name=f"xin{t}")
                        for c in range(3):
                            nc.sync.dma_start(
                                xin[:, c * 514:(c + 1) * 514],
                                xs[c, a:a + 128, :])
                        if g == 0 and k == 0:
                            nc.sync.dma_start(p24_b[:], p24[:, :])
                            nc.sync.dma_start(rvk_s[:], rvk[:, :])
                        tf = qf_p.tile([128, NQ], F32, tag="tf",
                                       name=f"tf{t}")
                        nc.gpsimd.tensor_scalar(
                            out=tf[:], in0=xin[:], scalar1=1.0,
                            scalar2=127.5, op0=OP.add, op1=OP.mult)
                        rf = qb_p.tile([128, NQ], BF16, tag="rf",
                                       name=f"rf{t}")
                        nc.gpsimd.tensor_scalar(
                            out=rf[:], in0=tf[:], scalar1=float(2 ** 23),
                            scalar2=float(2 ** 23), op0=OP.add,
                            op1=OP.subtract)
                        fixb = qb_p.tile([128, NQ], BF16, tag="fixb",
                                         name=f"fixb{t}")
                        nc.vector.tensor_tensor(out=fixb[:], in0=rf[:],
                                                in1=tf[:], op=OP.is_gt)
                        u = qb_p.tile([128, NQ], BF16, tag="u", name=f"u{t}")
                        nc.vector.tensor_tensor(out=u[:], in0=rf[:],
                                                in1=fixb[:], op=OP.subtract)
                        gxP = pgx.tile([128, NE], F32, tag="gxP",
                                       name=f"gxP{t}")
                        gyP = pgy.tile([128, NE], F32, tag="gyP",
                                       name=f"gyP{t}")
                        for c in range(3):
                            o = c * 514
                            d = gxP[:, c * 512:(c + 1) * 512]
                            nc.tensor.matmul(d, sob_b[:, 128:256],
                                             u[:, o:o + 512], start=True,
                                             stop=False)
                            nc.tensor.matmul(d, sob_b[:, 0:128],
                                             u[:, o + 2:o + 514],
                                             start=False, stop=True)
                            d = gyP[:, c * 512:(c + 1) * 512]
                            nc.tensor.matmul(d, sob_b[:, 256:384],
                                             u[:, o:o + 512], start=True,
                                             stop=False)
                            nc.tensor.matmul(d, sob_b[:, 384:512],
                                             u[:, o + 1:o + 513],
                                             start=False, stop=False)
                            nc.tensor.matmul(d, sob_b[:, 256:384],
                                             u[:, o + 2:o + 514],
                                             start=False, stop=True)
                        sl = slice(k * NE, (k + 1) * NE)
                        nc.scalar.copy(gxG[:, sl], gxP[:])
                        nc.scalar.copy(gyG[:, sl], gyP[:])
                        nc.scalar.activation(agxG[:, sl], gxP[:], AF.Abs)
                        nc.scalar.activation(agyG[:, sl], gyP[:], AF.Abs)

                    magcG = ev_p.tile([128, G * NE], I16, tag="magcG",
                                      name=f"magcG{g}")
                    nc.vector.tensor_tensor(out=magcG[:], in0=agxG[:],
                                            in1=agyG[:], op=OP.add)
                    # group views [128, G, .] per channel
                    mGv = magcG[:].rearrange("p (g n) -> p g n", g=G)
                    gxV = gxG[:].rearrange("p (g n) -> p g n", g=G)
                    gyV = gyG[:].rearrange("p (g n) -> p g n", g=G)
                    m0, m1, m2 = (mGv[:, :, c * 512:(c + 1) * 512]
                                  for c in range(3))
                    g0, g1, g2 = (gxV[:, :, c * 512:(c + 1) * 512]
                                  for c in range(3))
                    h0, h1, h2 = (gyV[:, :, c * 512:(c + 1) * 512]
                                  for c in range(3))
                    NS = G * 512
                    cmp01 = sml.tile([128, NS], I16, tag="T1",
                                     name=f"cmp01_{g}")
                    c01 = cmp01[:].rearrange("p (g n) -> p g n", g=G)
                    nc.vector.tensor_tensor(out=c01, in0=m0, in1=m1,
                                            op=OP.is_ge)
                    m01 = sml.tile([128, NS], I16, tag="T2", name=f"m01_{g}")
                    m01v = m01[:].rearrange("p (g n) -> p g n", g=G)
                    nc.vector.tensor_tensor(out=m01v, in0=m0, in1=m1,
                                            op=OP.max)
                    pick2 = sml.tile([128, NS], I16, tag="T3",
                                     name=f"pick2_{g}")
                    p2v = pick2[:].rearrange("p (g n) -> p g n", g=G)
                    nc.vector.tensor_tensor(out=p2v, in0=m2, in1=m01v,
                                            op=OP.is_gt)
                    gxs = sml.tile([128, NS], I16, tag="T4", name=f"gxs{g}")
                    gxsv = gxs[:].rearrange("p (g n) -> p g n", g=G)
                    nc.scalar.copy(gxsv, g1)
                    nc.vector.copy_predicated(gxsv, c01, g0)
                    nc.vector.copy_predicated(gxsv, p2v, g2)
                    gys = sml.tile([128, NS], I16, tag="T5", name=f"gys{g}")
                    gysv = gys[:].rearrange("p (g n) -> p g n", g=G)
                    nc.scalar.copy(gysv, h1)
                    nc.vector.copy_predicated(gysv, c01, h0)
                    nc.vector.copy_predicated(gysv, p2v, h2)
                    magp = mgp.tile([128, G * 514], I16, tag="magp",
                                    name=f"magp{g}")
                    mpv = magp[:].rearrange("p (g n) -> p g n", g=G)
                    nc.gpsimd.memset(magp[:], 0)
                    nc.vector.tensor_tensor(out=mpv[:, :, 1:513], in0=m01v,
                                            in1=m2, op=OP.max)
                    for bi_, t_ in ((0, 0), (1, T - 1)):
                        if t_ // G == g:
                            k_ = t_ % G
                            tmpb = mgp.tile([128, 514], I16, tag="tmpb",
                                            name=f"tmpb{g}")
                            nc.vector.tensor_tensor(
                                out=tmpb[:],
                                in0=magp[:, k_ * 514:(k_ + 1) * 514],
                                in1=rvk_s[:, bi_ * 514:(bi_ + 1) * 514],
                                op=OP.mult)
                            nc.vector.tensor_copy(
                                magp[:, k_ * 514:(k_ + 1) * 514], tmpb[:])
                    # sector masks
                    ax = sml.tile([128, NS], I16, tag="T1", name=f"ax{g}")
                    nc.scalar.activation(ax[:], gxs[:], AF.Abs)
                    ay = sml.tile([128, NS], I16, tag="T2", name=f"ay{g}")
                    nc.scalar.activation(ay[:], gys[:], AF.Abs)
                    hm = sml.tile([128, NS], I16, tag="T3", name=f"hm{g}")
                    nc.vector.scalar_tensor_tensor(
                        out=hm[:], in0=ax[:], scalar=TAN22, in1=ay[:],
                        op0=OP.mult, op1=OP.is_gt)
                    vm = sml.tile([128, NS], I16, tag="T6", name=f"vm{g}")
                    nc.vector.scalar_tensor_tensor(
                        out=vm[:], in0=ax[:], scalar=TAN67, in1=ay[:],
                        op0=OP.mult, op1=OP.is_lt)
                    pp = sml.tile([128, NS], BF16, tag="T7", name=f"pp{g}")
                    nc.vector.tensor_tensor(out=pp[:], in0=gxs[:],
                                            in1=gys[:], op=OP.mult)
                    ssm = sml.tile([128, NS], I16, tag="T4", name=f"ssm{g}")
                    nc.gpsimd.tensor_scalar(out=ssm[:], in0=pp[:],
                                            scalar1=0.0, scalar2=None,
                                            op0=OP.is_ge)
                    # neighbors via partition-shift DMA (whole group)
                    mu = mgp.tile([128, G * 514], I16, tag="mu",
                                  name=f"mu{g}")
                    nc.gpsimd.memset(mu[96:128, :], 0)
                    nc.sync.dma_start(mu[0:127, :], magp[1:128, :])
                    md = mgp.tile([128, G * 514], I16, tag="md",
                                  name=f"md{g}")
                    nc.gpsimd.memset(md[0:32, :], 0)
                    nc.sync.dma_start(md[1:128, :], magp[0:127, :])
                    mum1 = mgp.tile([128, G * 514], I16, tag="mum1",
                                    name=f"mum1_{g}")
                    nc.gpsimd.tensor_scalar(out=mum1[:], in0=mu[:],
                                            scalar1=1, scalar2=None,
                                            op0=OP.subtract)
                    mgm1 = mgp.tile([128, G * 514], I16, tag="mgm1",
                                    name=f"mgm1_{g}")
                    nc.gpsimd.tensor_scalar(out=mgm1[:], in0=magp[:],
                                            scalar1=1, scalar2=None,
                                            op0=OP.subtract)
                    muv = mu[:].rearrange("p (g n) -> p g n", g=G)
                    mdv = md[:].rearrange("p (g n) -> p g n", g=G)
                    mu1v = mum1[:].rearrange("p (g n) -> p g n", g=G)
                    mg1v = mgm1[:].rearrange("p (g n) -> p g n", g=G)
                    # keep = mag > max(nb, na-1); na-side uses >= via -1
                    M = sml.tile([128, NS], I16, tag="T5", name=f"M{g}")
                    Mv_ = M[:].rearrange("p (g n) -> p g n", g=G)
                    nc.vector.tensor_tensor(out=Mv_, in0=mdv[:, :, 2:514],
                                            in1=mu1v[:, :, 0:512], op=OP.max)
                    Md1 = sml.tile([128, NS], I16, tag="T7", name=f"Md1_{g}")
                    Md1v = Md1[:].rearrange("p (g n) -> p g n", g=G)
                    nc.vector.tensor_tensor(out=Md1v, in0=mdv[:, :, 0:512],
                                            in1=mu1v[:, :, 2:514], op=OP.max)
                    Mvv = sml.tile([128, NS], I16, tag="T8", name=f"Mvv{g}")
                    Mvvv = Mvv[:].rearrange("p (g n) -> p g n", g=G)
                    nc.vector.tensor_tensor(out=Mvvv, in0=mdv[:, :, 1:513],
                                            in1=mu1v[:, :, 1:513], op=OP.max)
                    Mh = sml.tile([128, NS], I16, tag="T9", name=f"Mh{g}")
                    Mhv = Mh[:].rearrange("p (g n) -> p g n", g=G)
                    nc.vector.tensor_tensor(out=Mhv, in0=mpv[:, :, 0:512],
                                            in1=mg1v[:, :, 2:514], op=OP.max)
                    nc.vector.copy_predicated(M[:], ssm[:], Md1[:])
                    nc.vector.copy_predicated(M[:], vm[:], Mvv[:])
                    nc.vector.copy_predicated(M[:], hm[:], Mh[:])
                    kc = sml.tile([128, NS], I16, tag="T1", name=f"kc{g}")
                    kcv = kc[:].rearrange("p (g n) -> p g n", g=G)
                    nc.vector.tensor_tensor(out=kcv, in0=mpv[:, :, 1:513],
                                            in1=Mv_, op=OP.is_gt)
                    km = sml.tile([128, NS], I16, tag="T2", name=f"km{g}")
                    kmv = km[:].rearrange("p (g n) -> p g n", g=G)
                    nc.vector.tensor_tensor(out=kmv, in0=mpv[:, :, 1:513],
                                            in1=kcv, op=OP.mult)
                    strong = swp.tile([128, NS], BF16, tag="strong",
                                      name=f"strong{g}")
                    nc.gpsimd.tensor_scalar(out=strong[:], in0=km[:],
                                            scalar1=200.0, scalar2=None,
                                            op0=OP.is_gt)
                    weak = swp.tile([128, NS], BF16, tag="weak",
                                    name=f"weak{g}")
                    nc.gpsimd.tensor_scalar(out=weak[:], in0=km[:],
                                            scalar1=100.0, scalar2=None,
                                            op0=OP.is_gt)
                    for k in range(G):
                        t = g * G + k
                        lhs = p24_b[:, t * WORDS:(t + 1) * WORDS]
                        ssl = slice(k * 512, (k + 1) * 512)
                        nc.tensor.matmul(mmS[:], lhs, strong[:, ssl],
                                         start=(t == 0), stop=(t == T - 1))
                        nc.tensor.matmul(mmW[:], lhs, weak[:, ssl],
                                         start=(t == 0), stop=(t == T - 1))

            # ============ phase B: packed hysteresis (word-local) ============
            with ExitStack() as bctx:
                nc.sync.dma_start(mrep_b[:], mrep[:, :])
                nc.sync.dma_start(patc_s[:], patc[:, :])
                hw_ = bctx.enter_context(tc.tile_pool(name="hw", bufs=1))
                it_p = bctx.enter_context(tc.tile_pool(name="itp", bufs=2))
                sW = hw_.tile([WORDS, 512], I32, tag="sW")
                nc.vector.tensor_copy(sW[:], mmW[:])
                cur = hw_.tile([WORDS, 512], I32, tag="cur0")
                nc.vector.tensor_copy(cur[:], mmS[:])
                for it in range(hyst_iters):
                    sl = it_p.tile([WORDS, 512], I32, tag="sl",
                                   name=f"sl{it}")
                    nc.vector.tensor_scalar(
                        out=sl[:], in0=cur[:], scalar1=1, scalar2=None,
                        op0=OP.logical_shift_left)
                    sr = it_p.tile([WORDS, 512], I32, tag="sr",
                                   name=f"sr{it}")
                    nc.vector.tensor_scalar(
                        out=sr[:], in0=cur[:], scalar1=1, scalar2=None,
                        op0=OP.logical_shift_right)
                    o1 = it_p.tile([WORDS, 512], I32, tag="o1",
                                   name=f"o1_{it}")
                    nc.vector.tensor_tensor(out=o1[:], in0=sl[:],
                                            in1=sr[:], op=OP.bitwise_or)
                    vor = it_p.tile([WORDS, 512], I32, tag="vor",
                                    name=f"vor{it}")
                    nc.vector.tensor_tensor(out=vor[:], in0=o1[:],
                                            in1=cur[:], op=OP.bitwise_or)
                    q = it_p.tile([WORDS, 512], I32, tag="q", name=f"q{it}")
                    nc.vector.tensor_tensor(
                        out=q[:, 1:512], in0=vor[:, 0:511],
                        in1=vor[:, 1:512], op=OP.bitwise_or)
                    nc.vector.tensor_copy(q[:, 0:1], vor[:, 0:1])
                    r = it_p.tile([WORDS, 512], I32, tag="r", name=f"r{it}")
                    nc.vector.tensor_tensor(
                        out=r[:, 0:511], in0=q[:, 0:511],
                        in1=vor[:, 1:512], op=OP.bitwise_or)
                    nc.vector.tensor_copy(r[:, 511:512], q[:, 511:512])
                    ncur = hw_.tile([WORDS, 512], I32, tag=f"cur{it + 1}",
                                    name=f"ncur{it + 1}")
                    nc.vector.tensor_tensor(out=ncur[:], in0=r[:],
                                            in1=sW[:], op=OP.bitwise_and)
                    cur = ncur
                bi = []
                for j, (s1v, s2v, o0, o1v) in enumerate([
                        (255, None, OP.bitwise_and, None),
                        (8, 255, OP.logical_shift_right, OP.bitwise_and),
                        (16, 255, OP.logical_shift_right, OP.bitwise_and),
                ]):
                    x_ = hw_.tile([WORDS, 512], I32, tag=f"bi{j}",
                                  name=f"bi{j}")
                    if o1v is None:
                        nc.vector.tensor_scalar(
                            out=x_[:], in0=cur[:], scalar1=s1v,
                            scalar2=None, op0=o0)
                    else:
                        nc.vector.tensor_scalar(
                            out=x_[:], in0=cur[:], scalar1=s1v,
                            scalar2=s2v, op0=o0, op1=o1v)
                    bi.append(x_)
                b0 = hw_.tile([WORDS, 512], BF16, tag="b0")
                nc.scalar.copy(b0[:], bi[0][:])
                b1 = hw_.tile([WORDS, 512], BF16, tag="b1")
                nc.scalar.copy(b1[:], bi[1][:])
                b2 = hw_.tile([WORDS, 512], BF16, tag="b2")
                nc.scalar.copy(b2[:], bi[2][:])
                unp = bctx.enter_context(
                    tc.tile_pool(name="unp", bufs=2, space="PSUM"))
                uo_p = bctx.enter_context(tc.tile_pool(name="uo", bufs=3))
                consts_host, w0s = make_consts(T, rows_out)
                for o in range(n_out):
                    w0 = w0s[o]
                    bs = [uo_p.tile([8, 512], BF16, tag=f"bs{j}",
                                    name=f"bs{j}_{o}")
                          for j in range(3)]
                    for j, bsrc in enumerate((b0, b1, b2)):
                        nc.sync.dma_start(bs[j][:], bsrc[w0:w0 + 8, :])
                    ps = unp.tile([128, 512], F32, tag="ps", name=f"ps{o}")
                    for j in range(3):
                        nc.tensor.matmul(
                            ps[:],
                            mrep_b[:, (o * 3 + j) * 128:(o * 3 + j + 1) * 128],
                            bs[j][:], start=(j == 0), stop=(j == 2))
                    pse = uo_p.tile([128, 512], I32, tag="pse",
                                    name=f"pse{o}")
                    nc.scalar.copy(pse[:], ps[:])
                    bits = uo_p.tile([128, 512], I32, tag="bits",
                                     name=f"bits{o}")
                    nc.vector.tensor_scalar(out=bits[:], in0=pse[:],
                                            scalar1=patc_s[:, o:o + 1],
                                            scalar2=None,
                                            op0=OP.bitwise_and)
                    ot = uo_p.tile([128, 512], F32, tag="ot", name=f"ot{o}")
                    nc.vector.tensor_scalar(out=ot[:], in0=bits[:],
                                            scalar1=0, scalar2=255.0,
                                            op0=OP.not_equal, op1=OP.mult)
                    nc.sync.dma_start(
                        out[o * OUT_TILE:(o + 1) * OUT_TILE, :], ot[:])

    nc.compile()
    return nc


# ---------------- host-side helpers ----------------

def shard_inputs(x, T=18, rows_out=2048, n_cores=8):
    B, C, H, W = x.shape
    NR = B * H
    WORDS = (STRIDE * T) // MPACK
    tall = np.ascontiguousarray(x.transpose(1, 0, 2, 3).reshape(C, NR, W))
    tallp = np.pad(tall, ((0, 0), (0, 0), (1, 1)), mode='edge')
    EXT = ext_rows(T)
    consts, _ = make_consts(T, rows_out)
    maps = []
    for k in range(n_cores):
        r0 = k * rows_out - 6
        idx = np.clip(np.arange(r0, r0 + EXT), 0, NR - 1)
        shard = np.ascontiguousarray(tallp[:, idx, :])
        # per-core row-validity for boundary tiles (tall row in [0, NR))
        rvk = np.ones((128, 2 * 514), np.int16)
        for bi, t in ((0, 0), (1, T - 1)):
            rows = r0 + STRIDE * t + np.arange(128)
            bad = (rows < 0) | (rows >= NR)
            rvk[bad, bi * 514:(bi + 1) * 514] = 0
        # per-core pack stationary: zero strip rows outside the image
        valid = np.zeros((T, 128), bool)
        for t in range(T):
            g = k * rows_out - 4 + STRIDE * t + (np.arange(128) - 2)
            valid[t] = (g >= 0) & (g < NR)
        p24 = make_p24(T, WORDS, valid)
        m = {"xs": shard, "rvk": rvk, "p24": p24.astype(BF)}
        m.update(consts)
        maps.append(m)
    return maps


def assemble_output(results, B=32, H=512, W=512):
    outs = [r["out"] for r in results]
    tallout = np.concatenate(outs, axis=0)
    img = tallout.reshape(B, H, W)
    return np.broadcast_to(img[:, None], (B, 3, H, W))


# ---------------- harness entry point ----------------

_NC_CACHE = {}


def _get_nc():
    if "nc" not in _NC_CACHE:
        _NC_CACHE["nc"] = build_canny(T=18, rows_out=2048, hyst_iters=3)
    return _NC_CACHE["nc"]


def kernel(x):
    """Full-input entry point: x (32,3,512,512) f32 -> (32,3,512,512) f32."""
    from concourse.bass_utils import run_bass_kernel_spmd
    x = np.asarray(x, dtype=np.float32)
    nc = _get_nc()
    in_maps = shard_inputs(x, T=18, rows_out=2048, n_cores=8)
    res = run_bass_kernel_spmd(nc, in_maps, list(range(8)))
    out = assemble_output(res.results)
    return np.ascontiguousarray(out).astype(np.float32)


# revision 18
# speedup vs baseline: 1.1031x; 1.0009x over previous
"""Canny edge-detection Bass kernel (per-core program), v3.

Geometry (per core):
  - Output rows: rows_out (2048) of the tall image, [R0, R0+rows_out).
  - Tile t reads input rows [120t, 120t+128) of the xs shard (xs row 0 is
    tall row R0-6); valid NMS rows on partitions p in [2, 122).
  - Tiles are processed in groups of G=3: elementwise stages run once per
    group on [128, G*N] tensors (strided 3-d APs per sub-tile), amortizing
    the per-instruction fixed cost (~131ns DVE / ~185ns Act / ~156ns Pool).

Pipeline (engine placement from measured cost-model + compiler support):
  - Quant (per tile): tf=(x+1)*127.5 (Pool), rf=round via 2^23 (Pool, bf16
    out), fixb=rf>tf (DVE mixed), u=rf-fixb (DVE bf16 2x) = floor exact.
  - Sobel fully on PE: gx = Sb@u[2]-Sb@u[0], gy = Sv@u[0]+2Sv@u[1]+Sv@u[2]
    accumulated in PSUM per channel; Act evacuates gx/gy (copy) and
    |gx|/|gy| (AF.Abs) into group tensors.
  - NMS: keep = mag > max(nb, na-1) (integer mags), sector maxes selected
    by 3 copy_predicated; km = mag*keep feeds both thresholds (Pool ts).
  - Hysteresis: 18 net rows + 3-bit margins per int32 word (24 bits, f32
    pack exact); 3 iterations are word-local (no partition-shift DMAs).
    Unpack with per-out-tile stationaries, OUT_TILE=128.
"""
import sys
sys.path.insert(0, '/opt/trn_rl_repo')
from contextlib import ExitStack
import numpy as np
import ml_dtypes

import concourse.bass as bass
import concourse.tile as tile
from concourse import bacc, mybir

F32 = mybir.dt.float32
BF16 = mybir.dt.bfloat16
I16 = mybir.dt.int16
I32 = mybir.dt.int32

OP = mybir.AluOpType
AF = mybir.ActivationFunctionType

TAN22 = 0.4142135623730951
TAN67 = 2.414213562373095

STRIDE = 120          # valid mask rows per tile
TILE_R = 128          # input rows per tile
MPACK = 18            # net rows per packed int32 word
MARG = 3              # margin bits each side of the net range
OUT_TILE = 128        # output rows per unpack tile
G = 3                 # tiles per elementwise group

BF = ml_dtypes.bfloat16


def ext_rows(T):
    return STRIDE * (T - 1) + TILE_R  # xs shard rows


def make_consts(T=18, rows_out=2048):
    WORDS = (STRIDE * T) // MPACK     # 2160/18 = 120
    n_out = rows_out // OUT_TILE      # 16
    # Sobel vertical stationaries, lhsT layout: out[m] = sum_k lhsT[k,m] u[k]
    # blocks: [S_blur | -S_blur | S_vd | 2*S_vd]
    sob = np.zeros((128, 512), np.float32)
    for m in range(128):
        sob[m, m] = 2.0
        if m - 1 >= 0:
            sob[m - 1, m] = 1.0
        if m + 1 < 128:
            sob[m + 1, m] = 1.0
        if m + 1 < 128:
            sob[m + 1, 256 + m] = 1.0
        if m - 1 >= 0:
            sob[m - 1, 256 + m] = -1.0
    sob[:, 128:256] = -sob[:, 0:128]
    sob[:, 384:512] = 2.0 * sob[:, 256:384]
    # margin-pack stationaries: strip row s lands in every word w where
    # bit b = s - 18w + 3 is in [0, 24); net bits are [3, 21).
    # (built per-core in shard_inputs to zero out-of-image rows)
    # unpack one-hots, per out-tile: partition p reads strip row
    # s = 4 + 128o + p -> word w = s//18 (8-row window from w0(o)),
    # bit b = s%18 + 3, byte j = b//8, in-byte bit k = b%8.
    mrep = np.zeros((8, n_out * 3 * 128), np.float32)
    patc = np.zeros((128, n_out), np.int32)
    w0s = []
    for o in range(n_out):
        w0 = (4 + OUT_TILE * o) // MPACK
        w0s.append(w0)
        for p in range(128):
            s = 4 + OUT_TILE * o + p
            w, b = s // MPACK, s % MPACK + MARG
            j, k = b // 8, b % 8
            assert 0 <= w - w0 < 8
            mrep[w - w0, (o * 3 + j) * 128 + p] = 1.0
            patc[p, o] = 1 << k
    return {"sob": sob.astype(BF), "mrep": mrep.astype(BF),
            "patc": patc}, w0s


def make_p24(T, WORDS, valid):
    """Pack stationary [128, T*WORDS]; valid[t, p] gates strip rows."""
    p24 = np.zeros((128, T * WORDS), np.float32)
    for t in range(T):
        for p in range(2, 122):
            if not valid[t, p]:
                continue
            s = STRIDE * t + (p - 2)
            for w in range(WORDS):
                b = s - MPACK * w + MARG
                if 0 <= b < MPACK + 2 * MARG:
                    p24[p, t * WORDS + w] = float(1 << b)
    return p24


def build_canny(T=18, rows_out=2048, hyst_iters=3):
    EXT = ext_rows(T)
    WORDS = (STRIDE * T) // MPACK
    assert STRIDE * T % MPACK == 0 and WORDS <= 128
    n_out = rows_out // OUT_TILE
    NG = T // G
    assert T % G == 0

    nc = bacc.Bacc("TRN2", target_bir_lowering=False, debug=False,
                   num_devices=8)
    xs = nc.dram_tensor("xs", [3, EXT, 514], F32, kind="ExternalInput").ap()
    sob = nc.dram_tensor("sob", [128, 512], BF16, kind="ExternalInput").ap()
    p24 = nc.dram_tensor("p24", [128, T * WORDS], BF16,
                         kind="ExternalInput").ap()
    mrep = nc.dram_tensor("mrep", [8, n_out * 3 * 128], BF16,
                          kind="ExternalInput").ap()
    patc = nc.dram_tensor("patc", [128, n_out], I32,
                          kind="ExternalInput").ap()
    rvk = nc.dram_tensor("rvk", [128, 2 * 514], I16,
                         kind="ExternalInput").ap()
    out = nc.dram_tensor("out", [rows_out, 512], F32,
                         kind="ExternalOutput").ap()

    NE = 3 * 512   # evac cols per tile
    NQ = 3 * 514   # quant cols per tile

    with tile.TileContext(nc) as tc:
        with ExitStack() as octx:
            cpool = octx.enter_context(tc.tile_pool(name="consts", bufs=1))
            sob_b = cpool.tile([128, 512], BF16, tag="sobb")
            nc.sync.dma_start(sob_b[:], sob[:, :])
            # remaining consts are DMA'd after tile 0's input loads (p24,
            # rvk) or at phase B start (mrep, patc) to keep the first
            # compute off the critical path.
            p24_b = cpool.tile([128, T * WORDS], BF16, tag="p24b")
            mrep_b = cpool.tile([8, n_out * 3 * 128], BF16, tag="mrepb")
            patc_s = cpool.tile([128, n_out], I32, tag="patcs")
            rvk_s = cpool.tile([128, 2 * 514], I16, tag="rvks")

            pk = octx.enter_context(
                tc.tile_pool(name="packps", bufs=1, space="PSUM"))
            mmS = pk.tile([WORDS, 512], F32, tag="mmS")
            mmW = pk.tile([WORDS, 512], F32, tag="mmW")

            # ============ phase A: grouped Sobel + NMS ============
            with ExitStack() as actx:
                xin_p = actx.enter_context(tc.tile_pool(name="xin", bufs=3))
                qf_p = actx.enter_context(tc.tile_pool(name="qf", bufs=2))
                qb_p = actx.enter_context(tc.tile_pool(name="qb", bufs=2))
                pgx = actx.enter_context(
                    tc.tile_pool(name="pgx", bufs=1, space="PSUM"))
                pgy = actx.enter_context(
                    tc.tile_pool(name="pgy", bufs=1, space="PSUM"))
                ev_p = actx.enter_context(tc.tile_pool(name="ev", bufs=2))
                mgp = actx.enter_context(tc.tile_pool(name="mgp", bufs=1))
                sml = actx.enter_context(tc.tile_pool(name="sml", bufs=1))
                swp = actx.enter_context(tc.tile_pool(name="swp", bufs=1))

                for g in range(NG):
                    gxG = ev_p.tile([128, G * NE], I16, tag="gxG",
                                    name=f"gxG{g}")
                    gyG = ev_p.tile([128, G * NE], I16, tag="gyG",
                                    name=f"gyG{g}")
                    agxG = ev_p.tile([128, G * NE], I16, tag="agxG",
                                     name=f"agxG{g}")
                    agyG = ev_p.tile([128, G * NE], I16, tag="agyG",
                                     name=f"agyG{g}")
                    for k in range(G):
                        t = g * G + k
                        a = STRIDE * t
                        xin = xin_p.tile([128, NQ], F32, tag="xin",
                                         name=f"xin{t}")
                        nc.sync.dma_start(
                            xin[:].rearrange("p (c w) -> p c w", c=3),
                            xs[:, a:a + 128, :].rearrange("c p w -> p c w"))
                        if g == 0 and k == 1:
                            nc.sync.dma_start(p24_b[:], p24[:, :])
                            nc.sync.dma_start(rvk_s[:], rvk[:, :])
                        tf = qf_p.tile([128, NQ], F32, tag="tf",
                                       name=f"tf{t}")
                        nc.gpsimd.tensor_scalar(
                            out=tf[:], in0=xin[:], scalar1=1.0,
                            scalar2=127.5, op0=OP.add, op1=OP.mult)
                        rf = qb_p.tile([128, NQ], BF16, tag="rf",
                                       name=f"rf{t}")
                        nc.gpsimd.tensor_scalar(
                            out=rf[:], in0=tf[:], scalar1=float(2 ** 23),
                            scalar2=float(2 ** 23), op0=OP.add,
                            op1=OP.subtract)
                        fixb = qb_p.tile([128, NQ], BF16, tag="fixb",
                                         name=f"fixb{t}")
                        nc.vector.tensor_tensor(out=fixb[:], in0=rf[:],
                                                in1=tf[:], op=OP.is_gt)
                        u = qb_p.tile([128, NQ], BF16, tag="u", name=f"u{t}")
                        nc.vector.tensor_tensor(out=u[:], in0=rf[:],
                                                in1=fixb[:], op=OP.subtract)
                        gxP = pgx.tile([128, NE], F32, tag="gxP",
                                       name=f"gxP{t}")
                        gyP = pgy.tile([128, NE], F32, tag="gyP",
                                       name=f"gyP{t}")
                        for c in range(3):
                            o = c * 514
                            d = gxP[:, c * 512:(c + 1) * 512]
                            nc.tensor.matmul(d, sob_b[:, 128:256],
                                             u[:, o:o + 512], start=True,
                                             stop=False)
                            nc.tensor.matmul(d, sob_b[:, 0:128],
                                             u[:, o + 2:o + 514],
                                             start=False, stop=True)
                            d = gyP[:, c * 512:(c + 1) * 512]
                            nc.tensor.matmul(d, sob_b[:, 256:384],
                                             u[:, o:o + 512], start=True,
                                             stop=False)
                            nc.tensor.matmul(d, sob_b[:, 384:512],
                                             u[:, o + 1:o + 513],
                                             start=False, stop=False)
                            nc.tensor.matmul(d, sob_b[:, 256:384],
                                             u[:, o + 2:o + 514],
                                             start=False, stop=True)
                        sl = slice(k * NE, (k + 1) * NE)
                        nc.scalar.copy(gxG[:, sl], gxP[:])
                        nc.scalar.copy(gyG[:, sl], gyP[:])
                        nc.scalar.activation(agxG[:, sl], gxP[:], AF.Abs)
                        nc.scalar.activation(agyG[:, sl], gyP[:], AF.Abs)

                    magcG = ev_p.tile([128, G * NE], I16, tag="magcG",
                                      name=f"magcG{g}")
                    nc.vector.tensor_tensor(out=magcG[:], in0=agxG[:],
                                            in1=agyG[:], op=OP.add)
                    # group views [128, G, .] per channel
                    mGv = magcG[:].rearrange("p (g n) -> p g n", g=G)
                    gxV = gxG[:].rearrange("p (g n) -> p g n", g=G)
                    gyV = gyG[:].rearrange("p (g n) -> p g n", g=G)
                    m0, m1, m2 = (mGv[:, :, c * 512:(c + 1) * 512]
                                  for c in range(3))
                    g0, g1, g2 = (gxV[:, :, c * 512:(c + 1) * 512]
                                  for c in range(3))
                    h0, h1, h2 = (gyV[:, :, c * 512:(c + 1) * 512]
                                  for c in range(3))
                    NS = G * 512
                    cmp01 = sml.tile([128, NS], I16, tag="T1",
                                     name=f"cmp01_{g}")
                    c01 = cmp01[:].rearrange("p (g n) -> p g n", g=G)
                    nc.vector.tensor_tensor(out=c01, in0=m0, in1=m1,
                                            op=OP.is_ge)
                    m01 = sml.tile([128, NS], I16, tag="T2", name=f"m01_{g}")
                    m01v = m01[:].rearrange("p (g n) -> p g n", g=G)
                    nc.vector.tensor_tensor(out=m01v, in0=m0, in1=m1,
                                            op=OP.max)
                    pick2 = sml.tile([128, NS], I16, tag="T3",
                                     name=f"pick2_{g}")
                    p2v = pick2[:].rearrange("p (g n) -> p g n", g=G)
                    nc.vector.tensor_tensor(out=p2v, in0=m2, in1=m01v,
                                            op=OP.is_gt)
                    gxs = sml.tile([128, NS], I16, tag="T4", name=f"gxs{g}")
                    gxsv = gxs[:].rearrange("p (g n) -> p g n", g=G)
                    nc.scalar.copy(gxsv, g1)
                    nc.vector.copy_predicated(gxsv, c01, g0)
                    nc.vector.copy_predicated(gxsv, p2v, g2)
                    gys = sml.tile([128, NS], I16, tag="T5", name=f"gys{g}")
                    gysv = gys[:].rearrange("p (g n) -> p g n", g=G)
                    nc.scalar.copy(gysv, h1)
                    nc.vector.copy_predicated(gysv, c01, h0)
                    nc.vector.copy_predicated(gysv, p2v, h2)
                    magp = mgp.tile([128, G * 514], I16, tag="magp",
                                    name=f"magp{g}")
                    mpv = magp[:].rearrange("p (g n) -> p g n", g=G)
                    nc.gpsimd.memset(magp[:], 0)
                    nc.vector.tensor_tensor(out=mpv[:, :, 1:513], in0=m01v,
                                            in1=m2, op=OP.max)
                    for bi_, t_ in ((0, 0), (1, T - 1)):
                        if t_ // G == g:
                            k_ = t_ % G
                            tmpb = mgp.tile([128, 514], I16, tag="tmpb",
                                            name=f"tmpb{g}")
                            nc.vector.tensor_tensor(
                                out=tmpb[:],
                                in0=magp[:, k_ * 514:(k_ + 1) * 514],
                                in1=rvk_s[:, bi_ * 514:(bi_ + 1) * 514],
                                op=OP.mult)
                            nc.vector.tensor_copy(
                                magp[:, k_ * 514:(k_ + 1) * 514], tmpb[:])
                    # sector masks
                    ax = sml.tile([128, NS], I16, tag="T1", name=f"ax{g}")
                    nc.scalar.activation(ax[:], gxs[:], AF.Abs)
                    ay = sml.tile([128, NS], I16, tag="T2", name=f"ay{g}")
                    nc.scalar.activation(ay[:], gys[:], AF.Abs)
                    hm = sml.tile([128, NS], I16, tag="T3", name=f"hm{g}")
                    nc.vector.scalar_tensor_tensor(
                        out=hm[:], in0=ax[:], scalar=TAN22, in1=ay[:],
                        op0=OP.mult, op1=OP.is_gt)
                    vm = sml.tile([128, NS], I16, tag="T6", name=f"vm{g}")
                    nc.vector.scalar_tensor_tensor(
                        out=vm[:], in0=ax[:], scalar=TAN67, in1=ay[:],
                        op0=OP.mult, op1=OP.is_lt)
                    pp = sml.tile([128, NS], BF16, tag="T7", name=f"pp{g}")
                    nc.vector.tensor_tensor(out=pp[:], in0=gxs[:],
                                            in1=gys[:], op=OP.mult)
                    ssm = sml.tile([128, NS], I16, tag="T4", name=f"ssm{g}")
                    nc.gpsimd.tensor_scalar(out=ssm[:], in0=pp[:],
                                            scalar1=0.0, scalar2=None,
                                            op0=OP.is_ge)
                    # neighbors via partition-shift DMA (whole group)
                    mu = mgp.tile([128, G * 514], I16, tag="mu",
                                  name=f"mu{g}")
                    nc.gpsimd.memset(mu[96:128, :], 0)
                    nc.sync.dma_start(mu[0:127, :], magp[1:128, :])
                    md = mgp.tile([128, G * 514], I16, tag="md",
                                  name=f"md{g}")
                    nc.gpsimd.memset(md[0:32, :], 0)
                    nc.sync.dma_start(md[1:128, :], magp[0:127, :])
                    mum1 = mgp.tile([128, G * 514], I16, tag="mum1",
                                    name=f"mum1_{g}")
                    nc.gpsimd.tensor_scalar(out=mum1[:], in0=mu[:],
                                            scalar1=1, scalar2=None,
                                            op0=OP.subtract)
                    mgm1 = mgp.tile([128, G * 514], I16, tag="mgm1",
                                    name=f"mgm1_{g}")
                    nc.gpsimd.tensor_scalar(out=mgm1[:], in0=magp[:],
                                            scalar1=1, scalar2=None,
                                            op0=OP.subtract)
                    muv = mu[:].rearrange("p (g n) -> p g n", g=G)
                    mdv = md[:].rearrange("p (g n) -> p g n", g=G)
                    mu1v = mum1[:].rearrange("p (g n) -> p g n", g=G)
                    mg1v = mgm1[:].rearrange("p (g n) -> p g n", g=G)
                    # keep = mag > max(nb, na-1); na-side uses >= via -1
                    M = sml.tile([128, NS], I16, tag="T5", name=f"M{g}")
                    Mv_ = M[:].rearrange("p (g n) -> p g n", g=G)
                    nc.vector.tensor_tensor(out=Mv_, in0=mdv[:, :, 2:514],
                                            in1=mu1v[:, :, 0:512], op=OP.max)
                    Md1 = sml.tile([128, NS], I16, tag="T7", name=f"Md1_{g}")
                    Md1v = Md1[:].rearrange("p (g n) -> p g n", g=G)
                    nc.vector.tensor_tensor(out=Md1v, in0=mdv[:, :, 0:512],
                                            in1=mu1v[:, :, 2:514], op=OP.max)
                    Mvv = sml.tile([128, NS], I16, tag="T8", name=f"Mvv{g}")
                    Mvvv = Mvv[:].rearrange("p (g n) -> p g n", g=G)
                    nc.vector.tensor_tensor(out=Mvvv, in0=mdv[:, :, 1:513],
                                            in1=mu1v[:, :, 1:513], op=OP.max)
                    Mh = sml.tile([128, NS], I16, tag="T9", name=f"Mh{g}")
                    Mhv = Mh[:].rearrange("p (g n) -> p g n", g=G)
                    nc.vector.tensor_tensor(out=Mhv, in0=mpv[:, :, 0:512],
                                            in1=mg1v[:, :, 2:514], op=OP.max)
                    nc.vector.copy_predicated(M[:], ssm[:], Md1[:])
                    nc.vector.copy_predicated(M[:], vm[:], Mvv[:])
                    nc.vector.copy_predicated(M[:], hm[:], Mh[:])
                    kc = sml.tile([128, NS], I16, tag="T1", name=f"kc{g}")
                    kcv = kc[:].rearrange("p (g n) -> p g n", g=G)
                    nc.vector.tensor_tensor(out=kcv, in0=mpv[:, :, 1:513],
                                            in1=Mv_, op=OP.is_gt)
                    km = sml.tile([128, NS], I16, tag="T2", name=f"km{g}")
                    kmv = km[:].rearrange("p (g n) -> p g n", g=G)
                    nc.vector.tensor_tensor(out=kmv, in0=mpv[:, :, 1:513],
                                            in1=kcv, op=OP.mult)
                    strong = swp.tile([128, NS], BF16, tag="strong",
                                      name=f"strong{g}")
                    nc.gpsimd.tensor_scalar(out=strong[:], in0=km[:],
                                            scalar1=200.0, scalar2=None,
                                            op0=OP.is_gt)
                    weak = swp.tile([128, NS], BF16, tag="weak",
                                    name=f"weak{g}")
                    nc.gpsimd.tensor_scalar(out=weak[:], in0=km[:],
                                            scalar1=100.0, scalar2=None,
                                            op0=OP.is_gt)
                    for k in range(G):
                        t = g * G + k
                        lhs = p24_b[:, t * WORDS:(t + 1) * WORDS]
                        ssl = slice(k * 512, (k + 1) * 512)
                        nc.tensor.matmul(mmS[:], lhs, strong[:, ssl],
                                         start=(t == 0), stop=(t == T - 1))
                        nc.tensor.matmul(mmW[:], lhs, weak[:, ssl],
                                         start=(t == 0), stop=(t == T - 1))

            # ============ phase B: packed hysteresis (word-local) ============
            with ExitStack() as bctx:
                nc.sync.dma_start(mrep_b[:], mrep[:, :])
                nc.sync.dma_start(patc_s[:], patc[:, :])
                hw_ = bctx.enter_context(tc.tile_pool(name="hw", bufs=1))
                it_p = bctx.enter_context(tc.tile_pool(name="itp", bufs=2))
                sW = hw_.tile([WORDS, 512], I32, tag="sW")
                nc.vector.tensor_copy(sW[:], mmW[:])
                cur = hw_.tile([WORDS, 512], I32, tag="cur0")
                nc.vector.tensor_copy(cur[:], mmS[:])
                for it in range(hyst_iters):
                    sl = it_p.tile([WORDS, 512], I32, tag="sl",
                                   name=f"sl{it}")
                    nc.vector.tensor_scalar(
                        out=sl[:], in0=cur[:], scalar1=1, scalar2=None,
                        op0=OP.logical_shift_left)
                    sr = it_p.tile([WORDS, 512], I32, tag="sr",
                                   name=f"sr{it}")
                    nc.vector.tensor_scalar(
                        out=sr[:], in0=cur[:], scalar1=1, scalar2=None,
                        op0=OP.logical_shift_right)
                    o1 = it_p.tile([WORDS, 512], I32, tag="o1",
                                   name=f"o1_{it}")
                    nc.vector.tensor_tensor(out=o1[:], in0=sl[:],
                                            in1=sr[:], op=OP.bitwise_or)
                    vor = it_p.tile([WORDS, 512], I32, tag="vor",
                                    name=f"vor{it}")
                    nc.vector.tensor_tensor(out=vor[:], in0=o1[:],
                                            in1=cur[:], op=OP.bitwise_or)
                    q = it_p.tile([WORDS, 512], I32, tag="q", name=f"q{it}")
                    nc.vector.tensor_tensor(
                        out=q[:, 1:512], in0=vor[:, 0:511],
                        in1=vor[:, 1:512], op=OP.bitwise_or)
                    nc.vector.tensor_copy(q[:, 0:1], vor[:, 0:1])
                    r = it_p.tile([WORDS, 512], I32, tag="r", name=f"r{it}")
                    nc.vector.tensor_tensor(
                        out=r[:, 0:511], in0=q[:, 0:511],
                        in1=vor[:, 1:512], op=OP.bitwise_or)
                    nc.vector.tensor_copy(r[:, 511:512], q[:, 511:512])
                    ncur = hw_.tile([WORDS, 512], I32, tag=f"cur{it + 1}",
                                    name=f"ncur{it + 1}")
                    nc.vector.tensor_tensor(out=ncur[:], in0=r[:],
                                            in1=sW[:], op=OP.bitwise_and)
                    cur = ncur
                bi = []
                for j, (s1v, s2v, o0, o1v) in enumerate([
                        (255, None, OP.bitwise_and, None),
                        (8, 255, OP.logical_shift_right, OP.bitwise_and),
                        (16, 255, OP.logical_shift_right, OP.bitwise_and),
                ]):
                    x_ = hw_.tile([WORDS, 512], I32, tag=f"bi{j}",
                                  name=f"bi{j}")
                    if o1v is None:
                        nc.vector.tensor_scalar(
                            out=x_[:], in0=cur[:], scalar1=s1v,
                            scalar2=None, op0=o0)
                    else:
                        nc.vector.tensor_scalar(
                            out=x_[:], in0=cur[:], scalar1=s1v,
                            scalar2=s2v, op0=o0, op1=o1v)
                    bi.append(x_)
                b0 = hw_.tile([WORDS, 512], BF16, tag="b0")
                nc.scalar.copy(b0[:], bi[0][:])
                b1 = hw_.tile([WORDS, 512], BF16, tag="b1")
                nc.scalar.copy(b1[:], bi[1][:])
                b2 = hw_.tile([WORDS, 512], BF16, tag="b2")
                nc.scalar.copy(b2[:], bi[2][:])
                unp = bctx.enter_context(
                    tc.tile_pool(name="unp", bufs=4, space="PSUM"))
                uo_p = bctx.enter_context(tc.tile_pool(name="uo", bufs=6))
                consts_host, w0s = make_consts(T, rows_out)
                for o in range(n_out):
                    w0 = w0s[o]
                    bs = [uo_p.tile([8, 512], BF16, tag=f"bs{j}",
                                    name=f"bs{j}_{o}")
                          for j in range(3)]
                    for j, bsrc in enumerate((b0, b1, b2)):
                        nc.sync.dma_start(bs[j][:], bsrc[w0:w0 + 8, :])
                    ps = unp.tile([128, 512], F32, tag="ps", name=f"ps{o}")
                    for j in range(3):
                        nc.tensor.matmul(
                            ps[:],
                            mrep_b[:, (o * 3 + j) * 128:(o * 3 + j + 1) * 128],
                            bs[j][:], start=(j == 0), stop=(j == 2))
                    pse = uo_p.tile([128, 512], I32, tag="pse",
                                    name=f"pse{o}")
                    nc.scalar.copy(pse[:], ps[:])
                    bits = uo_p.tile([128, 512], I32, tag="bits",
                                     name=f"bits{o}")
                    nc.vector.tensor_scalar(out=bits[:], in0=pse[:],
                                            scalar1=patc_s[:, o:o + 1],
                                            scalar2=None,
                                            op0=OP.bitwise_and)
                    ot = uo_p.tile([128, 512], F32, tag="ot", name=f"ot{o}")
                    nc.vector.tensor_scalar(out=ot[:], in0=bits[:],
                                            scalar1=0, scalar2=255.0,
                                            op0=OP.not_equal, op1=OP.mult)
                    nc.sync.dma_start(
                        out[o * OUT_TILE:(o + 1) * OUT_TILE, :], ot[:])

    nc.compile()
    return nc


# ---------------- host-side helpers ----------------

def shard_inputs(x, T=18, rows_out=2048, n_cores=8):
    B, C, H, W = x.shape
    NR = B * H
    WORDS = (STRIDE * T) // MPACK
    tall = np.ascontiguousarray(x.transpose(1, 0, 2, 3).reshape(C, NR, W))
    tallp = np.pad(tall, ((0, 0), (0, 0), (1, 1)), mode='edge')
    EXT = ext_rows(T)
    consts, _ = make_consts(T, rows_out)
    maps = []
    for k in range(n_cores):
        r0 = k * rows_out - 6
        idx = np.clip(np.arange(r0, r0 + EXT), 0, NR - 1)
        shard = np.ascontiguousarray(tallp[:, idx, :])
        # per-core row-validity for boundary tiles (tall row in [0, NR))
        rvk = np.ones((128, 2 * 514), np.int16)
        for bi, t in ((0, 0), (1, T - 1)):
            rows = r0 + STRIDE * t + np.arange(128)
            bad = (rows < 0) | (rows >= NR)
            rvk[bad, bi * 514:(bi + 1) * 514] = 0
        # per-core pack stationary: zero strip rows outside the image
        valid = np.zeros((T, 128), bool)
        for t in range(T):
            g = k * rows_out - 4 + STRIDE * t + (np.arange(128) - 2)
            valid[t] = (g >= 0) & (g < NR)
        p24 = make_p24(T, WORDS, valid)
        m = {"xs": shard, "rvk": rvk, "p24": p24.astype(BF)}
        m.update(consts)
        maps.append(m)
    return maps


def assemble_output(results, B=32, H=512, W=512):
    outs = [r["out"] for r in results]
    tallout = np.concatenate(outs, axis=0)
    img = tallout.reshape(B, H, W)
    return np.broadcast_to(img[:, None], (B, 3, H, W))


# ---------------- harness entry point ----------------

_NC_CACHE = {}


def _get_nc():
    if "nc" not in _NC_CACHE:
        _NC_CACHE["nc"] = build_canny(T=18, rows_out=2048, hyst_iters=3)
    return _NC_CACHE["nc"]


def kernel(x):
    """Full-input entry point: x (32,3,512,512) f32 -> (32,3,512,512) f32."""
    from concourse.bass_utils import run_bass_kernel_spmd
    x = np.asarray(x, dtype=np.float32)
    nc = _get_nc()
    in_maps = shard_inputs(x, T=18, rows_out=2048, n_cores=8)
    res = run_bass_kernel_spmd(nc, in_maps, list(range(8)))
    out = assemble_output(res.results)
    return np.ascontiguousarray(out).astype(np.float32)
